# revision 37
# baseline (speedup 1.0000x reference)
"""Trainium2 Bass kernel for nn_DynamycMoE (dense-masked top-2 MoE).

Strategy (MODE="ep"): expert-parallel in two SPMD launches.
  Phase 1 (data-parallel, fp32): each of the 8 cores computes top-2
  softmax gates for its 1024-token shard. Gating stays fully fp32: the
  top-2 *selection* is discontinuous, and bf16 logits flip selections on
  near-tied tokens, producing O(1) output errors. x-tile loads are split
  across the SP and Activation HWDGE queues so the transfers pipeline.
  Host dispatch: tokens are gathered per expert id (gate > 0), padded to
  a static capacity NCAP.
  Phase 2 (expert-parallel, bf16): core e runs expert e's MLP on its
  gathered tokens, activations token-on-free-axis:
      hT = relu(W1t.T @ xT + b1)        PSUM f32 -> SBUF bf16
      oT = W2t.T @ hT                   PSUM f32
      og = (oT + b2) * gate_bcast       -> SBUF bf16
      y  = og.T @ WmT                   PSUM f32 -> SBUF bf16 -> HBM
  bf16 matmuls run at 1 cycle/row (fp32 is 4) and halve DMA bytes; the
  smooth expert pipeline tolerates bf16 (rel err ~4.5e-3 vs the 2e-2
  gate; top-2 selection exactness is what matters and that is fp32).
  Host combines the two gated expert outputs per token in ascending
  expert order and applies the eps substitution.

Fallback (MODE="dp" or capacity overflow): fully-fused dense-masked MoE,
data-parallel over tokens (slow but always correct).
"""

import ml_dtypes
import numpy as np

import concourse.bacc as bacc
import concourse.bass as bass
import concourse.mybir as mybir
import concourse.tile as tile
from concourse import bass_utils

F32 = mybir.dt.float32
BF16 = mybir.dt.bfloat16
NP_BF16 = ml_dtypes.bfloat16
AF = mybir.ActivationFunctionType
ALU = mybir.AluOpType

B, D, H, E, C, T = 8192, 768, 256, 8, 64, 512
NCORES = 8
BL = B // NCORES  # tokens per core
TT = 256          # gate phase: token tile (free-dim) size
NT = BL // TT     # gate phase: token tiles per core
DC = D // 128     # K-chunks over D
HC = H // 128     # K-chunks over H
NPAIR = E // 2
EPS = float(np.finfo(np.float64).eps)

MODE = "ep"        # "ep": expert-parallel 2-phase; "dp": data-parallel dense
NCAP = 2176        # EP: padded per-expert capacity (seed-0 max load is 2162)
NCAP_BIG = 2304    # fallback capacity before resorting to dense DP
EP_DT = mybir.dt.bfloat16  # bf16: 1 cyc/row matmuls, half the DMA bytes
EP_NP = NP_BF16
TTE = 384          # EP: preferred token tile size


def _ep_tiles(ncap):
    """Token-tile widths for a capacity: full TTE tiles + one remainder."""
    tiles = [TTE] * (ncap // TTE)
    if ncap % TTE:
        tiles.append(ncap % TTE)
    assert all(t % 128 == 0 for t in tiles) and sum(tiles) == ncap
    return tiles


def _build_nc(reps=1):
    """Dense-masked data-parallel fallback (fp32, slow, always correct)."""
    nc = bacc.Bacc(
        "TRN2", target_bir_lowering=False, debug=False, enable_asserts=False
    )

    xT_h = nc.dram_tensor("xT", [128, NT * DC * TT], F32, kind="ExternalInput")
    wg_h = nc.dram_tensor("wg", [128, DC * E], F32, kind="ExternalInput")
    w1_h = nc.dram_tensor("w1", [128, E * DC * H], F32, kind="ExternalInput")
    b1_h = nc.dram_tensor("b1", [128, E * HC], F32, kind="ExternalInput")
    w2_h = nc.dram_tensor("w2", [128, E * HC * C], F32, kind="ExternalInput")
    b2_h = nc.dram_tensor("b2", [64, E], F32, kind="ExternalInput")
    wm_h = nc.dram_tensor("wm", [128, NPAIR * T], F32, kind="ExternalInput")
    id_h = nc.dram_tensor("ident", [128, 128], F32, kind="ExternalInput")
    y_h = nc.dram_tensor("y", [BL, T], F32, kind="ExternalOutput")

    w1_v = w1_h[:].rearrange("p (e c h) -> p e c h", e=E, c=DC)
    xT_v = xT_h[:].rearrange("p (i c t) -> p i c t", i=NT, c=DC)

    with tile.TileContext(nc) as tc:
        with (
            tc.tile_pool(name="weights", bufs=1) as wpool,
            tc.tile_pool(name="gates", bufs=1) as gpool,
            tc.tile_pool(name="gtmp", bufs=2) as gtmp,
            tc.tile_pool(name="hsb", bufs=3) as hpool,
            tc.tile_pool(name="og", bufs=3) as ogpool,
            tc.tile_pool(name="gb", bufs=4) as gbpool,
            tc.tile_pool(name="yout", bufs=4) as ypool,
        ):
            wg = wpool.tile([128, DC, E], F32, tag="wg")
            nc.sync.dma_start(wg[:], wg_h[:].rearrange("p (c e) -> p c e", c=DC))
            xts = []
            for ti in range(NT):
                xt = wpool.tile([128, DC, TT], F32, tag=f"x{ti}")
                nc.sync.dma_start(xt[:], xT_v[:, ti, :, :])
                xts.append(xt)
            w1s = []
            for e in range(E):
                w1e = wpool.tile([128, DC, H], F32, tag=f"w1_{e}")
                nc.sync.dma_start(w1e[:], w1_v[:, e, :, :])
                w1s.append(w1e)
            b1 = wpool.tile([128, E, HC], F32, tag="b1")
            nc.sync.dma_start(b1[:], b1_h[:].rearrange("p (e c) -> p e c", e=E))
            w2 = wpool.tile([128, E, HC, C], F32, tag="w2")
            nc.sync.dma_start(
                w2[:], w2_h[:].rearrange("p (e c k) -> p e c k", e=E, c=HC)
            )
            b2 = wpool.tile([64, E], F32, tag="b2")
            nc.sync.dma_start(b2[:], b2_h[:])
            wm = wpool.tile([128, NPAIR, T], F32, tag="wm")
            nc.sync.dma_start(wm[:], wm_h[:].rearrange("p (g t) -> p g t", g=NPAIR))
            ident = wpool.tile([128, 128], F32, tag="ident")
            nc.sync.dma_start(ident[:], id_h[:])

            for _ in range(reps):
                gflats = []
                with tc.tile_pool(
                    name="ps_gate", bufs=2, space=bass.MemorySpace.PSUM
                ) as ps_g:
                    for ti in range(NT):
                        gatesT = gpool.tile([8, TT], F32, tag=f"gatesT{ti}")
                        for qq in range(TT // 128):
                            tok = qq * 128
                            lg = ps_g.tile([128, E], F32, tag="lg")
                            for kc in range(DC):
                                nc.tensor.matmul(
                                    lg[:],
                                    xts[ti][:, kc, tok : tok + 128],
                                    wg[:, kc, :],
                                    start=(kc == 0),
                                    stop=(kc == DC - 1),
                                )
                            mx1 = gtmp.tile([128, 1], F32, tag="mx1")
                            nc.vector.reduce_max(
                                mx1[:], lg[:], axis=mybir.AxisListType.X
                            )
                            is1 = gtmp.tile([128, E], F32, tag="is1")
                            nc.vector.tensor_scalar(
                                is1[:], lg[:], mx1[:], None, op0=ALU.is_equal
                            )
                            masked = gtmp.tile([128, E], F32, tag="masked")
                            nc.vector.scalar_tensor_tensor(
                                masked[:],
                                is1[:],
                                -1e30,
                                lg[:],
                                op0=ALU.mult,
                                op1=ALU.add,
                            )
                            mx2 = gtmp.tile([128, 1], F32, tag="mx2")
                            nc.vector.reduce_max(
                                mx2[:], masked[:], axis=mybir.AxisListType.X
                            )
                            is2 = gtmp.tile([128, E], F32, tag="is2")
                            nc.vector.tensor_scalar(
                                is2[:], masked[:], mx2[:], None, op0=ALU.is_equal
                            )
                            d = gtmp.tile([128, 1], F32, tag="d")
                            nc.vector.tensor_sub(d[:], mx2[:], mx1[:])
                            ed = gtmp.tile([128, 1], F32, tag="ed")
                            nc.scalar.activation(ed[:], d[:], AF.Exp)
                            den = gtmp.tile([128, 1], F32, tag="den")
                            nc.vector.tensor_scalar_add(den[:], ed[:], 1.0)
                            g1 = gtmp.tile([128, 1], F32, tag="g1")
                            nc.vector.reciprocal(g1[:], den[:])
                            g2 = gtmp.tile([128, 1], F32, tag="g2")
                            nc.vector.tensor_mul(g2[:], ed[:], g1[:])
                            t2 = gtmp.tile([128, E], F32, tag="t2")
                            nc.vector.tensor_scalar_mul(t2[:], is2[:], g2[:])
                            gq = gtmp.tile([128, E], F32, tag="gq")
                            nc.vector.scalar_tensor_tensor(
                                gq[:], is1[:], g1[:], t2[:], op0=ALU.mult, op1=ALU.add
                            )
                            tr = ps_g.tile([8, 128], F32, tag="tr")
                            nc.tensor.transpose(tr[:], gq[:], ident[:])
                            nc.vector.tensor_copy(gatesT[:, tok : tok + 128], tr[:])
                        gflat = gpool.tile([1, E, TT], F32, tag=f"gflat{ti}")
                        nc.sync.dma_start(gflat[0:1, :, :], gatesT[:, :])
                        gflats.append(gflat)

                with (
                    tc.tile_pool(
                        name="ps_h", bufs=2, space=bass.MemorySpace.PSUM
                    ) as ps_h,
                    tc.tile_pool(
                        name="ps_o", bufs=2, space=bass.MemorySpace.PSUM
                    ) as ps_o,
                    tc.tile_pool(
                        name="ps_y", bufs=2, space=bass.MemorySpace.PSUM
                    ) as ps_y,
                ):
                    for ti in range(NT):
                        y_ps = ps_y.tile([128, TT // 128, T], F32, tag="y")
                        for pair in range(NPAIR):
                            og = ogpool.tile([128, TT], F32, tag="og")
                            for j in range(2):
                                e = 2 * pair + j
                                hT = ps_h.tile([128, HC, TT], F32, tag="h")
                                for half in range(HC):
                                    for kc in range(DC):
                                        nc.tensor.matmul(
                                            hT[:, half, :],
                                            w1s[e][:, kc, half * 128 : half * 128 + 128],
                                            xts[ti][:, kc, :],
                                            start=(kc == 0),
                                            stop=(kc == DC - 1),
                                        )
                                hs = hpool.tile([128, HC, TT], F32, tag="hs")
                                for half in range(HC):
                                    nc.scalar.activation(
                                        hs[:, half, :],
                                        hT[:, half, :],
                                        AF.Relu,
                                        bias=b1[:, e, half : half + 1],
                                    )
                                oT = ps_o.tile([64, TT], F32, tag="o")
                                for kc in range(HC):
                                    nc.tensor.matmul(
                                        oT[:],
                                        w2[:, e, kc, :],
                                        hs[:, kc, :],
                                        start=(kc == 0),
                                        stop=(kc == HC - 1),
                                    )
                                gb = gbpool.tile([64, TT], F32, tag="gb")
                                nc.gpsimd.partition_broadcast(
                                    gb[:], gflats[ti][0:1, e, :]
                                )
                                nc.vector.scalar_tensor_tensor(
                                    og[j * 64 : j * 64 + 64, :],
                                    oT[:],
                                    b2[:, e : e + 1],
                                    gb[:],
                                    op0=ALU.add,
                                    op1=ALU.mult,
                                )
                            for q in range(TT // 128):
                                nc.tensor.matmul(
                                    y_ps[:, q, :],
                                    og[:, q * 128 : q * 128 + 128],
                                    wm[:, pair, :],
                                    start=(pair == 0),
                                    stop=(pair == NPAIR - 1),
                                )
                        for q in range(TT // 128):
                            tok = ti * TT + q * 128
                            mask = ypool.tile([128, T], F32, tag="mask")
                            nc.vector.tensor_scalar(
                                mask[:], y_ps[:, q, :], 0.0, None, op0=ALU.is_equal
                            )
                            ysb = ypool.tile([128, T], F32, tag="ysb")
                            nc.vector.scalar_tensor_tensor(
                                ysb[:],
                                mask[:],
                                EPS,
                                y_ps[:, q, :],
                                op0=ALU.mult,
                                op1=ALU.add,
                            )
                            nc.sync.dma_start(y_h[tok : tok + 128, :], ysb[:])

    nc.compile()
    return nc


GW = 40  # stacked lhsT width: wg_hi in cols 0-7, wg_lo in cols 32-39
GCH = 2  # token chunk groups (512 tokens each -> one PSUM bank per group)


def _build_gate_nc(reps=1):
    """EP phase 1: per-core logits in expert-major layout, fp16 hi/lo split.

    The token-major [128tok, 8] gating layout costs 48 tiny matmuls at
    ~400ns of fixed per-instruction cost each (~19us on HW). Instead:
    lhsT = [wg_hi | pad | wg_lo] (fp16, 40 cols), rhs = x_hi / x_lo chunks
    (fp16, 512 tokens free) -> 24 big matmuls. All four cross products
    accumulate in fp32 PSUM, so logits are exact to ~2.4e-6 (the fp16
    residual |x - hi - lo| <= 2^-22|x|); the 2e-3-scale top-2 selection
    ties are resolved on the host, which recomputes near-tie tokens in
    exact fp32. Device output is just the [8, 1024] logit block; host does
    top-2 + softmax (dispatch bookkeeping, like the gather/combine).
    """
    CHT = BL // GCH  # tokens per chunk group (512)
    nc = bacc.Bacc(
        "TRN2", target_bir_lowering=False, debug=False, enable_asserts=False
    )
    FP16 = mybir.dt.float16
    xhl_h = nc.dram_tensor("xhl", [128, 2 * DC * BL], FP16, kind="ExternalInput")
    wgs_h = nc.dram_tensor("wgs", [128, DC * GW], FP16, kind="ExternalInput")
    # [8, hilo, chunk, t]: the wg_hi and wg_lo partial logits; host adds
    # them (DVE cannot read two PSUM operands, and the host sums for free).
    g_h = nc.dram_tensor("logits", [8, 2 * GCH * (BL // GCH)], F32,
                         kind="ExternalOutput")
    xhl_v = xhl_h[:].rearrange("p (h c t) -> p h c t", h=2, c=DC)

    with tile.TileContext(nc) as tc:
        with (
            tc.tile_pool(name="weights", bufs=1) as wpool,
            tc.tile_pool(name="gtmp", bufs=2) as gtmp,
            tc.tile_pool(name="ps_g", bufs=2, space=bass.MemorySpace.PSUM) as ps_g,
        ):
            wgs = wpool.tile([128, DC, GW], FP16, tag="wgs")
            nc.sync.dma_start(
                wgs[:], wgs_h[:].rearrange("p (c w) -> p c w", c=DC)
            )
            # x_hi streams on the SP queue, x_lo on the Act queue (which has
            # no activation instructions here, so no act-table load delays
            # it), each in kc-thirds so the kc-major matmul loop starts at
            # first-third arrival and stays just ahead of the stream.
            xhl = wpool.tile([128, 2, DC, BL], FP16, tag="xhl")
            for h, eng in ((0, nc.sync), (1, nc.scalar)):
                for a in range(0, DC, 2):
                    eng.dma_start(
                        xhl[:, h, a : a + 2], xhl_v[:, h, a : a + 2, :]
                    )

            # PE p-state prewarm during the DMA lead-in (see _build_ep_nc)
            warm = wpool.tile([128, 128], FP16, tag="warm")
            nc.gpsimd.memset(warm[:], 0)
            with tc.tile_pool(
                name="ps_warm", bufs=1, space=bass.MemorySpace.PSUM
            ) as ps_w:
                wps = ps_w.tile([128, 128], F32, tag="wps")
                for _ in range(16):
                    nc.tensor.matmul(
                        wps[:], warm[:], warm[:], start=True, stop=True
                    )

            for _ in range(reps):
                lg0 = ps_g.tile([GW, CHT], F32, tag="lg0")
                lg1 = ps_g.tile([GW, CHT], F32, tag="lg1")
                lgps = [lg0, lg1]
                for kc in range(DC):
                    for h in range(2):
                        for c in range(GCH):
                            nc.tensor.matmul(
                                lgps[c][:],
                                wgs[:, kc, :],
                                xhl[:, h, kc, c * CHT : (c + 1) * CHT],
                                start=(kc == 0 and h == 0),
                                stop=(kc == DC - 1 and h == 1),
                            )
                lgs = gtmp.tile([8, 2, GCH, CHT], F32, tag="lgs")
                for c in range(GCH):
                    nc.vector.tensor_copy(lgs[:, 0, c, :], lgps[c][0:8, :])
                    nc.vector.tensor_copy(lgs[:, 1, c, :], lgps[c][32:40, :])
                nc.sync.dma_start(
                    g_h[:].rearrange("p (h c t) -> p h c t", h=2, c=GCH), lgs[:]
                )
    nc.compile()
    return nc


def _build_ep_nc(reps=1, ncap=None):
    """EP phase 2 (bf16): one expert per core over ncap gathered tokens.

    All matmul operands are bf16 (1 cycle/row vs fp32's 4); PSUM stays
    f32. Inputs stream on the SP queue in first-use order; per-tile
    outputs are batched into one SWDGE DMA on the Pool queue. PSUM->bf16
    converts are split across the Activation and Vector engines so no
    single engine exceeds the PE's per-tile budget.
    """
    ncap = ncap or NCAP
    tiles = _ep_tiles(ncap)
    ntiles = len(tiles)
    toks = [sum(tiles[:i]) for i in range(ntiles)]        # token offsets
    xoff = [DC * t for t in toks]                          # xg free offsets
    total_x = DC * ncap
    nc = bacc.Bacc(
        "TRN2", target_bir_lowering=False, debug=False, enable_asserts=False
    )
    EPT = EP_DT
    xg_h = nc.dram_tensor("xg", [128, total_x], EPT, kind="ExternalInput")
    w1_h = nc.dram_tensor("w1", [128, HC * DC * 128], EPT, kind="ExternalInput")
    b1_h = nc.dram_tensor("b1", [128, HC], F32, kind="ExternalInput")
    w2_h = nc.dram_tensor("w2", [128, HC * C], EPT, kind="ExternalInput")
    b2_h = nc.dram_tensor("b2", [64, 1], F32, kind="ExternalInput")
    wm_h = nc.dram_tensor("wm", [64, T], EPT, kind="ExternalInput")
    gr_h = nc.dram_tensor("grow", [1, ncap], F32, kind="ExternalInput")
    yp_h = nc.dram_tensor("yp", [ncap, T], EPT, kind="ExternalOutput")
    w1_v = w1_h[:].rearrange("p (f c h) -> p f c h", f=HC, c=DC)

    with tile.TileContext(nc) as tc:
        with (
            tc.tile_pool(name="weights", bufs=1) as wpool,
            tc.tile_pool(name="hsb", bufs=5) as hpool,
            tc.tile_pool(name="og", bufs=5) as ogpool,
            tc.tile_pool(name="gb", bufs=3) as gbpool,
            tc.tile_pool(name="yout", bufs=5) as ypool,
            tc.tile_pool(name="ps_h", bufs=2, space=bass.MemorySpace.PSUM) as ps_h,
            tc.tile_pool(name="ps_o", bufs=2, space=bass.MemorySpace.PSUM) as ps_o,
            tc.tile_pool(name="ps_y", bufs=2, space=bass.MemorySpace.PSUM) as ps_y,
        ):
            # SP queue carries the critical-path stream in first-use order:
            # W1 half-0 slab, xg tile 0, W1 half-1, then grouped xg tiles
            # (grouping amortizes the ~650ns per-DMA DGE bubble). The small
            # params ride the Activation queue and all land before first use.
            # Act queue: W1 (both halves, one transfer) then the small
            # params, all done by ~4us. SP queue: the xg token stream, tile 0
            # split in kc-halves so the first fc1 matmul starts at ~3us.
            w1all = wpool.tile([128, HC, DC, 128], EPT, tag="w1")
            xgall = wpool.tile([128, total_x], EPT, tag="xg")
            xgs = [
                xgall[:, xoff[i] : xoff[i] + DC * tiles[i]].rearrange(
                    "p (c t) -> p c t", c=DC
                )
                for i in range(ntiles)
            ]
            nc.sync.dma_start(w1all[:], w1_v[:])
            def xg_dma(a, b, eng):  # free-range DMA
                eng.dma_start(xgall[:, a:b], xg_h[:, a:b])
            half0 = DC // 2 * tiles[0]
            xg_dma(0, half0, nc.sync)
            xg_dma(half0, xoff[1], nc.sync)
            xg_dma(xoff[1], xoff[2], nc.sync)
            xg_dma(xoff[2], xoff[4], nc.sync)
            xg_dma(xoff[4], total_x, nc.sync)
            w1h = [w1all[:, half] for half in range(HC)]

            b1 = wpool.tile([128, HC], F32, tag="b1")
            nc.scalar.dma_start(b1[:], b1_h[:])
            grow = wpool.tile([1, ncap], F32, tag="grow")
            nc.scalar.dma_start(grow[:], gr_h[:])
            w2 = wpool.tile([128, HC, C], EPT, tag="w2")
            nc.scalar.dma_start(w2[:], w2_h[:].rearrange("p (c k) -> p c k", c=HC))
            b2 = wpool.tile([64, 1], F32, tag="b2")
            nc.scalar.dma_start(b2[:], b2_h[:])
            wm = wpool.tile([64, T], EPT, tag="wm")
            nc.scalar.dma_start(wm[:], wm_h[:])

            # PE p-state prewarm: dependent dummy matmuls on a zeroed tile
            # keep the PE busy through the input-DMA lead-in so the 3us
            # frequency ramp (0.65 -> 1.2 -> 2.4 GHz) completes before the
            # first real matmul. Results land in a scratch PSUM tile that
            # is never read.
            warm = wpool.tile([128, 128], EPT, tag="warm")
            nc.gpsimd.memset(warm[:], 0)
            wps = ps_h.tile([128, TTE], F32, tag="h0")
            for _ in range(18):
                nc.tensor.matmul(
                    wps[:, 0:128], warm[:], warm[:], start=True, stop=True
                )

            # whole-capacity gate broadcast, once: keeps the Pool engine out
            # of the per-tile dependency chain (its queue also carries the
            # batched output DMAs).
            gball = gbpool.tile([64, ncap], F32, tag="gball")
            nc.gpsimd.partition_broadcast(gball[:], grow[0:1, :])

            for rr in range(reps):
                # software pipeline, depth 3: PE iteration t issues
                # fc1(t), fc2(t-1), mapper(t-2) so the Act relu and the DVE
                # og op each get a full tile period of slack before the PE
                # consumes their output.
                def emit_map(ti, og):
                    qt = tiles[ti] // 128
                    ysb = ypool.tile([128, qt, T], EPT, tag="ysb")
                    last = ti == ntiles - 1
                    for q in range(qt):
                        y_ps = ps_y.tile([128, T], F32, tag="y")
                        nc.tensor.matmul(
                            y_ps[:],
                            og[:, q * 128 : q * 128 + 128],
                            wm[:],
                            start=True,
                            stop=True,
                        )
                        # spread the PSUM->bf16 converts across Act and DVE
                        if (q + ti) % 2 == 0:
                            nc.scalar.copy(ysb[:, q, :], y_ps[:])
                        else:
                            nc.vector.tensor_copy(ysb[:, q, :], y_ps[:])
                        if last:
                            # final tile: per-chunk writes on three queues so
                            # the tail is one small transfer, not a batched
                            # SWDGE generation + many-descriptor DMA.
                            eng = (nc.gpsimd, nc.sync, nc.scalar)[q % 3]
                            tok = toks[ti] + q * 128
                            eng.dma_start(
                                yp_h[tok : tok + 128, :], ysb[:, q, :]
                            )
                    if not last:
                        # alternate output queues: Pool's SWDGE generation
                        # (~1.1us per batched DMA) would serialize behind
                        # the per-tile cadence on its own.
                        eng = nc.gpsimd if ti % 2 == 0 else nc.sync
                        eng.dma_start(
                            yp_h[toks[ti] : toks[ti] + tiles[ti], :].rearrange(
                                "(q p) t -> p q t", p=128
                            ),
                            ysb[:],
                        )

                def emit_fc1(ti):
                    tte = tiles[ti]
                    hs_t = hpool.tile([128, HC, TTE], EPT, tag="hs")
                    hT0_t = ps_h.tile([128, TTE], F32, tag="h0")
                    hT1_t = ps_h.tile([128, TTE], F32, tag="h1")
                    hs = hs_t[:, :, 0:tte]
                    hTs = [hT0_t[:, 0:tte], hT1_t[:, 0:tte]]
                    # interleave the two half-chains so PE always has an
                    # independent matmul to issue between dependent
                    # accumulation steps.
                    for kc in range(DC):
                        for half in range(HC):
                            nc.tensor.matmul(
                                hTs[half][:],
                                w1h[half][:, kc, :],
                                xgs[ti][:, kc, :],
                                start=(kc == 0),
                                stop=(kc == DC - 1),
                            )
                    for half in range(HC):
                        nc.scalar.activation(
                            hs[:, half, :],
                            hTs[half][:],
                            AF.Relu,
                            bias=b1[:, half : half + 1],
                        )
                    return hs

                def emit_fc2(ti, hs):
                    tte = tiles[ti]
                    oT_t = ps_o.tile([64, TTE], F32, tag="o")
                    oT = oT_t[:, 0:tte]
                    for kc in range(HC):
                        nc.tensor.matmul(
                            oT[:],
                            w2[:, kc, :],
                            hs[:, kc, :],
                            start=(kc == 0),
                            stop=(kc == HC - 1),
                        )
                    og_t = ogpool.tile([64, TTE], EPT, tag="og")
                    og = og_t[:, 0:tte]
                    nc.vector.scalar_tensor_tensor(
                        og[:],
                        oT[:],
                        b2[:, 0:1],
                        gball[:, toks[ti] : toks[ti] + tte],
                        op0=ALU.add,
                        op1=ALU.mult,
                    )
                    return og

                hs_q, og_q = [], []
                for ti in range(ntiles):
                    hs_q.append((ti, emit_fc1(ti)))
                    if len(hs_q) > 1:
                        t2i, hs2 = hs_q.pop(0)
                        og_q.append((t2i, emit_fc2(t2i, hs2)))
                    if len(og_q) > 1:
                        emit_map(*og_q.pop(0))
                for t2i, hs2 in hs_q:
                    og_q.append((t2i, emit_fc2(t2i, hs2)))
                for item in og_q:
                    emit_map(*item)
    nc.compile()
    return nc


_NC_CACHE = {}


def _get_nc(which="dp"):
    if which not in _NC_CACHE:
        _NC_CACHE[which] = {
            "dp": _build_nc,
            "gate": _build_gate_nc,
            "ep": _build_ep_nc,
            "ep_big": lambda: _build_ep_nc(ncap=NCAP_BIG),
        }[which]()
    return _NC_CACHE[which]


def _host_prep(x, w_gate, W1, b1, W2, b2, Wm):
    """Dense fallback: rearrange weights into SBUF images; shard x."""
    f = np.float32
    xs = []
    for c in range(NCORES):
        s = x[c * BL : (c + 1) * BL]  # [BL, D]
        img = np.ascontiguousarray(
            s.reshape(NT, TT, DC, 128).transpose(3, 0, 2, 1).reshape(128, -1)
        )
        xs.append(img)
    W1t = W1.transpose(0, 2, 1)  # [E, D, H]
    w1_img = np.ascontiguousarray(
        W1t.reshape(E, DC, 128, H).transpose(2, 0, 1, 3).reshape(128, -1)
    )
    W2t = W2.transpose(0, 2, 1)  # [E, H, C]
    w2_img = np.ascontiguousarray(
        W2t.reshape(E, HC, 128, C).transpose(2, 0, 1, 3).reshape(128, -1)
    )
    WmT = Wm.transpose(0, 2, 1)  # [E, C, T]
    wm_img = np.ascontiguousarray(
        WmT.reshape(NPAIR, 128, T).transpose(1, 0, 2).reshape(128, -1)
    )
    wg_img = np.ascontiguousarray(
        w_gate.reshape(DC, 128, E).transpose(1, 0, 2).reshape(128, -1)
    )
    b1_img = np.ascontiguousarray(
        b1.reshape(E, HC, 128).transpose(2, 0, 1).reshape(128, -1)
    )
    b2_img = np.ascontiguousarray(b2.T)  # [C, E]
    ident = np.eye(128, dtype=f)
    shared = {
        "wg": wg_img.astype(f, copy=False),
        "w1": w1_img.astype(f, copy=False),
        "b1": b1_img.astype(f, copy=False),
        "w2": w2_img.astype(f, copy=False),
        "b2": b2_img.astype(f, copy=False),
        "wm": wm_img.astype(f, copy=False),
        "ident": ident,
    }
    return [dict(shared, xT=xs[c].astype(f, copy=False)) for c in range(NCORES)]


def _x_images(x):
    """Per-core feature-major fp32 SBUF images (dense DP fallback)."""
    xs = []
    for c in range(NCORES):
        s = x[c * BL : (c + 1) * BL]
        xs.append(
            np.ascontiguousarray(
                s.reshape(NT, TT, DC, 128).transpose(3, 0, 2, 1).reshape(128, -1)
            )
        )
    return xs


def _wg_image(w_gate):
    return np.ascontiguousarray(
        w_gate.reshape(DC, 128, E).transpose(1, 0, 2).reshape(128, -1)
    )


def _gate_maps(x, w_gate):
    """Per-core fp16 hi/lo gate-phase in_maps.

    xhl image [p, hl, kc, t] = split(x)[hl][token t, kc*128+p];
    wgs image [p, kc, 0:8]=wg_hi, [p, kc, 32:40]=wg_lo.
    """
    f16 = np.float16
    wg_hi = w_gate.astype(f16)
    wg_lo = (w_gate - wg_hi.astype(np.float32)).astype(f16)
    wgs = np.zeros((DC, 128, GW), f16)
    wgs[:, :, 0:E] = wg_hi.reshape(DC, 128, E)
    wgs[:, :, 32 : 32 + E] = wg_lo.reshape(DC, 128, E)
    wgs_img = np.ascontiguousarray(wgs.transpose(1, 0, 2).reshape(128, -1))
    x_hi = x.astype(f16)
    x_lo = (x - x_hi.astype(np.float32)).astype(f16)
    maps = []
    for c in range(NCORES):
        sl = slice(c * BL, (c + 1) * BL)
        img = np.stack(
            [
                h[sl].reshape(BL, DC, 128).transpose(2, 1, 0)  # [128, kc, t]
                for h in (x_hi, x_lo)
            ],
            axis=1,
        )  # [128, 2, DC, BL]
        maps.append(
            {
                "xhl": np.ascontiguousarray(img.reshape(128, -1)),
                "wgs": wgs_img,
            }
        )
    return maps


def _host_gates(logits, x, w_gate, tie_tol=1e-3):
    """Top-2 softmax gates from device logits; near-tie tokens recomputed
    in exact fp32 so the selected expert SET always matches the reference
    (device logits are exact to ~2.4e-6; any token whose top2/top3 gap is
    below tie_tol gets its logits recomputed on the host)."""
    srt = np.sort(logits, axis=1)
    amb = np.flatnonzero(srt[:, -2] - srt[:, -3] < tie_tol)
    if amb.size:
        logits[amb] = x[amb] @ w_gate
    idx = np.argsort(-logits, axis=1)[:, :2]
    v = np.take_along_axis(logits, idx, axis=1)
    sm = np.exp(v - v.max(axis=1, keepdims=True))
    sm /= sm.sum(axis=1, keepdims=True)
    gates = np.zeros_like(logits)
    np.put_along_axis(gates, idx, sm.astype(np.float32), axis=1)
    return gates


def _make_ep_map(xg, grow, W1e, b1e, W2e, b2e, Wme):
    """Build the bf16 phase-2 in_map for one expert.

    xg: [ncap, D] f32 (gathered+padded tokens), grow: [1, ncap] f32.
    Image layout per token tile: [p, kc, t] slabs concatenated.
    """
    bf = EP_NP
    ncap = xg.shape[0]
    slabs = []
    off = 0
    for tte in _ep_tiles(ncap):
        s = xg[off : off + tte]  # [tte, D]
        slabs.append(
            s.reshape(tte, DC, 128).transpose(2, 1, 0).reshape(128, -1)
        )
        off += tte
    xg_img = np.ascontiguousarray(np.concatenate(slabs, axis=1)).astype(bf)
    # half-major [p, half, kc, h'] to match the split W1 slab tiles
    w1_img = np.ascontiguousarray(
        W1e.T.reshape(DC, 128, HC, 128).transpose(1, 2, 0, 3).reshape(128, -1)
    ).astype(bf)
    w2_img = np.ascontiguousarray(
        W2e.T.reshape(HC, 128, C).transpose(1, 0, 2).reshape(128, -1)
    ).astype(bf)
    return {
        "xg": xg_img,
        "w1": w1_img,
        "b1": np.ascontiguousarray(b1e.reshape(HC, 128).T),
        "w2": w2_img,
        "b2": np.ascontiguousarray(b2e[:, None]),
        "wm": np.ascontiguousarray(Wme.T).astype(bf),  # [C, T]
        "grow": grow,
    }


def _kernel_ep(x, w_gate, W1, b1, W2, b2, Wm):
    # phase 1: on-device logits (fp16 hi/lo split, fp32-exact), host top-2
    g_maps = _gate_maps(x, w_gate)
    nc_g = _get_nc("gate")
    res_g = bass_utils.run_bass_kernel_spmd(nc_g, g_maps, list(range(NCORES)))
    logits = np.concatenate(
        [
            res_g.results[c]["logits"].reshape(E, 2, BL).sum(axis=1).T
            for c in range(NCORES)
        ],
        axis=0,
    )  # [B, E]; device rows are [e, hilo, chunk*t]
    gates = _host_gates(logits, x, w_gate)

    # host dispatch: gather tokens per expert (top-2 membership = gate > 0)
    idxs = [np.flatnonzero(gates[:, e] > 0.0) for e in range(E)]
    maxload = max(i.size for i in idxs)
    if maxload <= NCAP:
        ncap, which = NCAP, "ep"
    elif maxload <= NCAP_BIG:
        ncap, which = NCAP_BIG, "ep_big"
    else:
        return None  # pathological imbalance -> caller uses dense DP
    ep_maps = []
    for e in range(E):
        idx = idxs[e]
        xg = np.zeros((ncap, D), np.float32)
        xg[: idx.size] = x[idx]
        grow = np.zeros((1, ncap), np.float32)
        grow[0, : idx.size] = gates[idx, e]
        ep_maps.append(_make_ep_map(xg, grow, W1[e], b1[e], W2[e], b2[e], Wm[e]))

    # phase 2: one expert per core
    nc_e = _get_nc(which)
    res_e = bass_utils.run_bass_kernel_spmd(nc_e, ep_maps, list(range(NCORES)))

    # combine on host (expert-ascending order, matching the reference sum)
    y = np.zeros((B, T), np.float32)
    for e in range(E):
        y[idxs[e]] += res_e.results[e]["yp"][: idxs[e].size].astype(np.float32)
    y[y == 0.0] = np.float32(EPS)
    return y


def kernel(x, labels, w_gate, W1, b1, W2, b2, Wm, _trace=False):
    x = np.asarray(x, dtype=np.float32)
    w_gate = np.asarray(w_gate, np.float32)
    W1 = np.asarray(W1, np.float32)
    b1 = np.asarray(b1, np.float32)
    W2 = np.asarray(W2, np.float32)
    b2 = np.asarray(b2, np.float32)
    Wm = np.asarray(Wm, np.float32)
    if MODE == "ep":
        y = _kernel_ep(x, w_gate, W1, b1, W2, b2, Wm)
        if y is not None:
            return y
        # pathological expert load imbalance: use the dense DP kernel
    in_maps = _host_prep(x, w_gate, W1, b1, W2, b2, Wm)
    nc = _get_nc()
    res = bass_utils.run_bass_kernel_spmd(
        nc, in_maps, list(range(NCORES)), trace=_trace
    )
    y = np.concatenate([res.results[c]["y"] for c in range(NCORES)], axis=0)
    if _trace:
        kernel.last_results = res
    return y
